# revision 42
# baseline (speedup 1.0000x reference)
"""Trainium2 Bass kernel for batched multi-head attention (B=8, N=M=C=1024,
H=16, D=64), data-parallel across 8 NeuronCores (one batch element per core).

v3: scheduling overhaul of v2 targeting PE continuity (p-state) and queue
latency:
  - startup DMAs round-robin across the sync/gpsimd/scalar sequencers in
    consumption order (first projection operands first); first real matmul
    ~7us instead of ~26us; warmup trimmed 24->12 matmuls.
  - QK regrouped to one m-chunk x 2 heads per PSUM tile (2 banks/group) with
    the score ring deepened to bufs=3 (6 banks): ~3 groups in flight so the
    PE rides ahead of the scalar-engine exp without stalling on PSUM reuse.
  - khTz zero quadrants memset once up front on the DVE (removes 16 gpsimd
    memsets and their queue-ordering hazard with startup DMA issue).
  - target-mask multiply batched: one [128,2,512] DVE op per group with the
    mask operand broadcast (stride-0) across the two heads.
  - den broadcast matmuls ride the score ring (one [P,2,512] tile, both
    heads) freeing 2 PSUM banks for the deeper score ring; norm_fin placed
    ~3 groups after norm_den so the PE never waits on the scalar denb copy.

Per-core dataflow (bf16 matmul inputs, f32 PSUM accumulate): identical math
to v2 -- projections with head-transposed layouts, QK^T with zero-padded
K=128 lhsT, exp on the scalar engine with 1/sqrt(D) folded, target-mask as
DVE multiply, AV with a 65th indicator column producing the softmax
denominator, K=1 ones-matmul broadcast + fast reciprocal normalize,
o-projection with bias as a K=1 ones matmul.
"""
import sys

sys.path.insert(0, "/opt/trn_rl_repo")

import numpy as np
import ml_dtypes

import concourse.bass as bass  # noqa: F401
import concourse.mybir as mybir
import concourse.bacc as bacc
import concourse.tile as tile
from concourse import bass_utils

B = 8
N = 1024   # queries
M = 1024   # keys
C = 1024   # model dim
H = 16
D = 64
NP = 8     # head pairs
P = 128
NB = 2     # n blocks of 512
SCALE = D ** -0.5

F32 = mybir.dt.float32
F32R = mybir.dt.float32r
BF16 = mybir.dt.bfloat16
I32 = mybir.dt.int32
MUL = mybir.AluOpType.mult
EXP = mybir.ActivationFunctionType.Exp
NPBF = ml_dtypes.bfloat16

_NC_CACHE = {}


def build_nc():
    nc = bacc.Bacc("TRN2", target_bir_lowering=False, debug=False, num_devices=1)

    qbT_d = nc.dram_tensor("qbT", [P, 8, N], BF16, kind="ExternalInput").ap()
    kbT_d = nc.dram_tensor("kbT", [P, 8, M], BF16, kind="ExternalInput").ap()
    vbT_d = nc.dram_tensor("vbT", [P, 8, M], BF16, kind="ExternalInput").ap()
    tmT_d = nc.dram_tensor("tmT", [P, 8, N], BF16, kind="ExternalInput").ap()
    mb_d = nc.dram_tensor("maskb", [P, 8], BF16, kind="ExternalInput").ap()
    eye_d = nc.dram_tensor("eye", [P, P], BF16, kind="ExternalInput").ap()
    wq_d = nc.dram_tensor("wq", [NP, P, 8, P], BF16, kind="ExternalInput").ap()
    wk_d = nc.dram_tensor("wk", [NP, P, 8, P], BF16, kind="ExternalInput").ap()
    wv_d = nc.dram_tensor("wv", [4, P, 8, 256], BF16, kind="ExternalInput").ap()
    wo_d = nc.dram_tensor("wo", [NP, P, C], BF16, kind="ExternalInput").ap()
    bob_d = nc.dram_tensor("bob", [P, C], BF16, kind="ExternalInput").ap()
    out_d = nc.dram_tensor("out", [N, C], BF16, kind="ExternalOutput").ap()

    with tile.TileContext(nc) as tc:
        _body(tc, nc, qbT_d, kbT_d, vbT_d, tmT_d, mb_d, eye_d, wq_d, wk_d,
              wv_d, wo_d, bob_d, out_d)
    nc.compile()
    return nc


def _body(tc, nc, qbT_d, kbT_d, vbT_d, tmT_d, mb_d, eye_d, wq_d, wk_d, wv_d,
          wo_d, bob_d, out_d):
    from contextlib import ExitStack
    ctx = ExitStack()
    with ctx:
        persist = ctx.enter_context(tc.tile_pool(name="persist", bufs=1))
        wpool = ctx.enter_context(tc.tile_pool(name="wpool", bufs=2))
        ptpool = ctx.enter_context(tc.tile_pool(name="ptpool", bufs=8))
        xpool = ctx.enter_context(tc.tile_pool(name="xpool", bufs=2))
        opool = ctx.enter_context(tc.tile_pool(name="opool", bufs=2))
        spsum = ctx.enter_context(tc.tile_pool(name="spsum", bufs=3, space="PSUM"))
        avpsum = ctx.enter_context(tc.tile_pool(name="avpsum", bufs=2, space="PSUM"))

        # ---- persistent SBUF tensors ----
        qbT = persist.tile([P, 8, N], BF16)   # [p, cc, n] = q[n, cc*128+p]
        kbT = persist.tile([P, 8, M], BF16)
        vbT = persist.tile([P, 8, M], BF16)   # key-masked v, transposed
        tmT = persist.tile([P, 8, N], BF16)   # [p, mc, n] = tmask[n, mc*128+p]
        qhT = persist.tile([P, NP, N], BF16)  # [p, j, n] = qh[n, j*128+p]
        # khTz[:, j, 0, m]: rows 0:64 = kh^T head 2j, rows 64:128 = 0
        # khTz[:, j, 1, m]: rows 0:64 = 0, rows 64:128 = kh^T head 2j+1
        # Zero-padding makes every QK^T matmul a uniform K=128 config (same
        # PE tile config as the projections). Zero quadrants are written once
        # by a whole-tensor DVE memset below; the k-evacuations only fill the
        # nonzero quadrants.
        khTz = persist.tile([P, NP, 2, M], BF16)
        vha = persist.tile([P, NP, 8, 130], BF16)
        xn = persist.tile([P, NP, N], BF16)   # [p, j, n] = x_norm[n, j*128+p]
        wob = persist.tile([P, NP, C], BF16)  # [p, j, c2] = Wo[j*128+p, c2]
        maskb = persist.tile([P, 8], BF16)
        eyeb = persist.tile([P, P], BF16)  # identity: tail-chain PSUM reload
        bob = persist.tile([P, C], BF16)  # row 0 = bias, rows 1.. = 0
        # ones-row constants: lhsT of K-padded broadcast matmuls. Using a
        # 128-row lhsT keeps every matmul in the kernel at the same
        # (128, 128) PE tile config.
        onesr0 = persist.tile([P, P], BF16)   # row 0 = 1, rest 0
        nc.vector.memset(onesr0[:], 0.0)
        nc.vector.memset(onesr0[0:1, :], 1.0)
        wu = persist.tile([P, 512], BF16)
        nc.vector.memset(wu[:], 0.0)
        # j=0's khTz zero quadrants on the (otherwise idle) DVE during the
        # startup DMA window; j>=1 quadrants are zeroed lazily on gpsimd in
        # the k-projection fillers (the gpsimd queue is clear mid-loop but
        # backed up ~20us at startup behind the phase-1/2 DMA issue).
        nc.vector.memset(khTz[64:128, 0, 0, :], 0.0)
        nc.vector.memset(khTz[0:64, 0, 1, :], 0.0)

        # PE warmup: ramp the clock domain while the first DMAs land.
        wups = spsum.tile([P, 2, 512], F32, tag="sp", name="wups")
        for i in range(12):
            nc.tensor.matmul(wups[:, i % 2, :], wu[:, 0:P], wu[:],
                             start=True, stop=True)

        # ---- startup DMAs ----
        # Phase 1 (critical path: first q/k projection chains): j=0 weights +
        # the nb0 halves of q/k, round-robin across the three DMA-capable
        # sequencers (sync, gpsimd, scalar). A single sequencer takes ~600ns
        # per dma_start, so 24 descriptors on one queue would be ~14us of
        # issue latency; on three it's ~5us.
        wqb0 = wpool.tile([P, 8, P], BF16, tag="wq")
        wkb0 = wpool.tile([P, 8, P], BF16, tag="wk")
        wvb0 = wpool.tile([P, 8, 256], BF16, tag="wv")
        phase1 = []
        for i in range(4):
            cs = slice(2 * i, 2 * i + 2)
            phase1.append((wqb0[:, cs, :], wq_d[0, :, cs, :]))
            phase1.append((qbT[:, 2 * i, 0:512], qbT_d[:, 2 * i, 0:512]))
            phase1.append((qbT[:, 2 * i + 1, 0:512], qbT_d[:, 2 * i + 1, 0:512]))
        for i in range(4):
            cs = slice(2 * i, 2 * i + 2)
            phase1.append((wkb0[:, cs, :], wk_d[0, :, cs, :]))
            phase1.append((kbT[:, 2 * i, 0:512], kbT_d[:, 2 * i, 0:512]))
            phase1.append((kbT[:, 2 * i + 1, 0:512], kbT_d[:, 2 * i + 1, 0:512]))
        engs3 = [nc.sync, nc.gpsimd, nc.scalar]
        for i, (dst, src) in enumerate(phase1):
            engs3[i % 3].dma_start(out=dst, in_=src)
        # Preload the Exp activation table during the DMA window instead of
        # paying ~1.3us at the first real exp. Emitted AFTER the phase-1
        # descriptors (so it doesn't delay their issue on the scalar queue)
        # and targeting maskb (whose own DMA lands later and overwrites it)
        # rather than wu, to avoid a WAR wait on the warmup matmuls.
        nc.scalar.activation(maskb[0:1, 0:1], wu[0:1, 0:1], EXP, scale=1.0)

        # Phase 2, in consumption order: nb0 masks (first tm-mult ~14us),
        # nb1 q/k (the j=0 nb1 projection filler ~15us), Wv + the m-lower
        # half of v (first v-projection chain ~16us), the rest of q/k/v,
        # nb1 masks (~22us), then the stragglers. The first 12 descriptors
        # still use all three sequencers (the scalar queue clears well
        # before the first exp); the rest alternate sync/gpsimd.
        # j=1's q/k weights are prefetched here too: proj(1) chains are j=0's
        # cheapest early fillers (0.5 MB of weights buys 6.8us of PE work),
        # letting the 2 MB of vbT land later without stalling the PE.
        wqb1 = wpool.tile([P, 8, P], BF16, tag="wq")
        wkb1 = wpool.tile([P, 8, P], BF16, tag="wk")
        phase2 = [(maskb[:], mb_d)]
        for m2 in range(4):
            ms2 = slice(2 * m2, 2 * m2 + 2)
            phase2.append((tmT[:, ms2, 0:512], tmT_d[:, ms2, 0:512]))
        for cc in range(8):
            phase2.append((qbT[:, cc, 512:1024], qbT_d[:, cc, 512:1024]))
            phase2.append((kbT[:, cc, 512:1024], kbT_d[:, cc, 512:1024]))
        for i in range(4):
            cs = slice(2 * i, 2 * i + 2)
            phase2.append((wqb1[:, cs, :], wq_d[1, :, cs, :]))
            phase2.append((wkb1[:, cs, :], wk_d[1, :, cs, :]))
        for i in range(4):
            cs = slice(2 * i, 2 * i + 2)
            phase2.append((wvb0[:, cs, :], wv_d[0, :, cs, :]))
            phase2.append((vbT[:, 2 * i, 0:512], vbT_d[:, 2 * i, 0:512]))
            phase2.append((vbT[:, 2 * i + 1, 0:512], vbT_d[:, 2 * i + 1, 0:512]))
        for cc in range(8):
            phase2.append((vbT[:, cc, 512:1024], vbT_d[:, cc, 512:1024]))
        for m2 in range(4):
            ms2 = slice(2 * m2, 2 * m2 + 2)
            phase2.append((tmT[:, ms2, 512:1024], tmT_d[:, ms2, 512:1024]))
        phase2.append((wob[:, 0, :], wo_d[0]))
        phase2.append((wob[:, 1, :], wo_d[1]))
        phase2.append((bob[:], bob_d))
        phase2.append((eyeb[:], eye_d))
        engs2 = [nc.sync, nc.gpsimd]
        for i, (dst, src) in enumerate(phase2):
            if i < 18:
                engs3[i % 3].dma_start(out=dst, in_=src)
            else:
                engs2[i % 2].dma_start(out=dst, in_=src)

        def load_weights(j):
            """Weights for iteration j>=1, alternating sync/gpsimd."""
            wqb = wpool.tile([P, 8, P], BF16, tag="wq")
            wkb = wpool.tile([P, 8, P], BF16, tag="wk")
            descs = []
            for i in range(4):
                cs = slice(2 * i, 2 * i + 2)
                descs.append((wqb[:, cs, :], wq_d[j, :, cs, :]))
                descs.append((wkb[:, cs, :], wk_d[j, :, cs, :]))
            wvb = None
            if j % 2 == 0:
                wvb = wpool.tile([P, 8, 256], BF16, tag="wv")
                for i in range(4):
                    cs = slice(2 * i, 2 * i + 2)
                    descs.append((wvb[:, cs, :], wv_d[j // 2, :, cs, :]))
            descs.append((wob[:, j, :], wo_d[j]))
            for i, (dst, src) in enumerate(descs):
                engs2[i % 2].dma_start(out=dst, in_=src)
            return wqb, wkb, wvb

        # ---- helpers ----
        def proj_pair(j, nb, wqb, wkb):
            """q+k projection chains for (j, nb): 16 K=128 matmuls sharing
            one score-ring tile (emitted adjacently so the ring slot is
            held open only briefly), plus the PSUM->SBUF evacuations."""
            def run():
                sp = spsum.tile([P, 2, 512], F32, tag="sp", name=f"pj{j}_{nb}")
                ns = slice(nb * 512, (nb + 1) * 512)
                for cc in range(8):
                    nc.tensor.matmul(sp[:, 0, :], wqb[:, cc, :],
                                     qbT[:, cc, ns],
                                     start=(cc == 0), stop=(cc == 7))
                nc.vector.tensor_copy(qhT[:, j, ns], sp[:, 0, :])
                if nb == 0 and j > 0:
                    nc.gpsimd.memset(khTz[64:128, j, 0, :], 0.0)
                    nc.gpsimd.memset(khTz[0:64, j, 1, :], 0.0)
                for cc in range(8):
                    nc.tensor.matmul(sp[:, 1, :], wkb[:, cc, :],
                                     kbT[:, cc, ns],
                                     start=(cc == 0), stop=(cc == 7))
                nc.vector.tensor_copy(khTz[0:64, j, 0, ns], sp[0:64, 1, :])
                nc.vector.tensor_copy(khTz[64:128, j, 1, ns], sp[64:128, 1, :])
            return run

        def make_vproj_fillers(j, wvb):
            """v projection chains for pair (j, j+1), one filler per 2 m-chunks."""
            tiles = {}

            def chain(mg, mi2):
                def run():
                    if mg not in tiles:
                        tiles[mg] = spsum.tile([P, 2, 512], F32, tag="sp",
                                               name=f"pv{j}_{mg}")
                    pvv = tiles[mg].rearrange("p a b -> p (a b)").rearrange(
                        "p (m d) -> p m d", m=4)
                    if mg == 0 and mi2 == 0:
                        # indicator columns first (maskb lands early): every
                        # av chain reads them, so they must not wait for the
                        # last v-chain.
                        for jx in (j, j + 1):
                            nc.vector.tensor_copy(vha[:, jx, :, 64], maskb[:])
                            nc.vector.tensor_copy(vha[:, jx, :, 129], maskb[:])
                    for mi in (mi2, mi2 + 1):
                        mc = mg * 4 + mi
                        ms = slice(mc * P, (mc + 1) * P)
                        for cc in range(8):
                            nc.tensor.matmul(pvv[:, mi, :], vbT[:, cc, ms],
                                             wvb[:, cc, :],
                                             start=(cc == 0), stop=(cc == 7))
                        out_sl = vha[:, j:j + 2, mc, :].rearrange(
                            "p j (hx dd) -> p j hx dd", hx=2)[:, :, :, 0:64]
                        in_sl = pvv[:, mi, :].rearrange(
                            "p (j hx dd) -> p j hx dd", j=2, hx=2)
                        nc.vector.tensor_copy(out_sl, in_sl)
                return run

            return [chain(0, 0), chain(0, 2), chain(1, 0), chain(1, 2)]

        def qk_group(j, nb, mc, pts):
            """Scores for one m-chunk x 2 heads: 2 matmuls, 1 exp, 1 tm-mult."""
            ns = slice(nb * 512, (nb + 1) * 512)
            ms = slice(mc * P, (mc + 1) * P)
            sp = spsum.tile([P, 2, 512], F32, tag="sp")
            nc.tensor.matmul(sp[:, 0, :], khTz[:, j, 0, ms], qhT[:, j, ns],
                             start=True, stop=True)
            nc.tensor.matmul(sp[:, 1, :], khTz[:, j, 1, ms], qhT[:, j, ns],
                             start=True, stop=True)
            pt = ptpool.tile([P, 2, 512], BF16, tag="pt")
            nc.scalar.activation(pt[:], sp[:], EXP, scale=SCALE)
            tsl = tmT[:, mc, ns][:, None, :].broadcast_to([P, 2, 512])
            nc.vector.tensor_tensor(pt[:], pt[:], tsl, MUL)
            pts[mc] = pt

        av_tiles = {}

        def av_half(j, nb, pts, half):
            """AV numerator+denominator accumulation, m-chunks half*4..half*4+3."""
            if half == 0:
                av_tiles[nb] = (avpsum.tile([65, 512], F32, tag="av", name="av0"),
                                avpsum.tile([65, 512], F32, tag="av", name="av1"))
            av0, av1 = av_tiles[nb]
            for mc in range(half * 4, half * 4 + 4):
                nc.tensor.matmul(av0[:], vha[:, j, mc, 0:65], pts[mc][:, 0, :],
                                 start=(mc == 0), stop=(mc == 7))
            for mc in range(half * 4, half * 4 + 4):
                nc.tensor.matmul(av1[:], vha[:, j, mc, 65:130], pts[mc][:, 1, :],
                                 start=(mc == 0), stop=(mc == 7))

        onesr64 = persist.tile([P, P], BF16)  # row 64 = 1, rest 0
        nc.vector.memset(onesr64[:], 0.0)
        nc.vector.memset(onesr64[64:65, :], 1.0)

        def norm_den(nb):
            """Phase 1: denominator rows -> bf16 SBUF. On the DVE (gpsimd
            cannot touch PSUM): its queue is shallower than the scalar
            engine's exp backlog, so norm_fin's bc matmuls aren't stuck
            behind head-of-line exps."""
            av0, av1 = av_tiles[nb]
            dens = []
            for av in (av0, av1):
                denb = xpool.tile([65, 512], BF16, tag="denb", bufs=4)
                nc.vector.tensor_copy(denb[:], av[:])
                dens.append(denb)
            return dens

        def norm_fin(j, nb, dens):
            """Phase 2: K=65 ones-matmul broadcast + reciprocal + multiply.
            One [P,2,512] score-ring tile holds both heads' broadcasts."""
            ns = slice(nb * 512, (nb + 1) * 512)
            av0, av1 = av_tiles[nb]
            bc2 = spsum.tile([P, 2, 512], F32, tag="sp", name="bc")
            nc.tensor.matmul(bc2[:, 0, :], onesr64[0:65, :],
                             dens[0][:], start=True, stop=True)
            nc.tensor.matmul(bc2[:, 1, :], onesr64[0:65, :],
                             dens[1][:], start=True, stop=True)
            for hx, av in enumerate((av0, av1)):
                rc = xpool.tile([64, 512], F32, tag="rc")
                nc.vector.reciprocal_approx_fast(rc[:], bc2[0:64, hx, :])
                rows = slice(0, 64) if hx == 0 else slice(64, 128)
                nc.vector.tensor_tensor(xn[rows, j, ns], av[0:64, :], rc[:], MUL)

        def oproj_evac_dma(nch, c2h, po, evac):
            ot = opool.tile([P, 512], BF16, tag="ot")
            if evac is nc.scalar:
                nc.scalar.copy(ot[:], po[:, 0, :])
            else:
                evac.tensor_copy(ot[:], po[:, 0, :])
            nsl = slice(nch * P, (nch + 1) * P)
            c0 = c2h * 512
            eng = nc.sync if (nch + c2h) % 2 == 0 else nc.gpsimd
            eng.dma_start(out=out_d[nsl, c0:c0 + 512], in_=ot[:])

        def oproj_chain(nch, c2h, evac=None):
            nsl = slice(nch * P, (nch + 1) * P)
            c2s = slice(c2h * 512, (c2h + 1) * 512)
            po = spsum.tile([P, 2, 512], F32, tag="sp", name="po")
            nc.tensor.matmul(po[:, 0, :], onesr0[:], bob[:, c2s],
                             start=True, stop=False)
            for jj in range(NP):
                nc.tensor.matmul(po[:, 0, :], xn[:, jj, nsl], wob[:, jj, c2s],
                                 start=False, stop=(jj == NP - 1))
            # bf16 output (upcast on host): halves the tail's DRAM write.
            oproj_evac_dma(nch, c2h, po, evac or nc.scalar)

        def oproj_partial(nch, c2h, store):
            """Tail chain, part 1: bias + head-pairs 0..6, evacuated to a
            bf16 staging slot. Runs as PE filler during j=7's nb1 phase so
            only a 2-matmul finish remains after the last normalize."""
            nsl = slice(nch * P, (nch + 1) * P)
            c2s = slice(c2h * 512, (c2h + 1) * 512)
            po = spsum.tile([P, 2, 512], F32, tag="sp", name="po")
            nc.tensor.matmul(po[:, 0, :], onesr0[:], bob[:, c2s],
                             start=True, stop=False)
            for jj in range(NP - 1):
                nc.tensor.matmul(po[:, 0, :], xn[:, jj, nsl], wob[:, jj, c2s],
                                 start=False, stop=(jj == NP - 2))
            nc.vector.tensor_copy(store, po[:, 0, :])

        def oproj_finish(nch, c2h, store):
            """Tail chain, part 2: identity-reload of the bf16 partial plus
            the head-pair-7 contribution."""
            nsl = slice(nch * P, (nch + 1) * P)
            c2s = slice(c2h * 512, (c2h + 1) * 512)
            po = spsum.tile([P, 2, 512], F32, tag="sp", name="po")
            nc.tensor.matmul(po[:, 0, :], eyeb[:], store,
                             start=True, stop=False)
            nc.tensor.matmul(po[:, 0, :], xn[:, NP - 1, nsl],
                             wob[:, NP - 1, c2s], start=False, stop=True)
            oproj_evac_dma(nch, c2h, po, nc.scalar)

        # ---- software-pipelined main loop ----
        # Pre-loop: only the nb0 projections for j=0; the nb1 chains and the
        # v-projections run as j=0's fillers so the PE is not stalled on
        # later-arriving DMAs.
        proj_pair(0, 0, wqb0, wkb0)()

        pending = None  # (j, dens) for the deferred nb1 normalize
        for j in range(NP):
            fillers = []
            if j == 0:
                fillers.append(proj_pair(0, 1, wqb0, wkb0))
                fillers.append(proj_pair(1, 0, wqb1, wkb1))
                fillers.append(proj_pair(1, 1, wqb1, wkb1))
                fillers += make_vproj_fillers(0, wvb0)
            elif j + 1 < NP:
                wqb_n, wkb_n, wvb_n = load_weights(j + 1)
                fillers.append(proj_pair(j + 1, 0, wqb_n, wkb_n))
                fillers.append(proj_pair(j + 1, 1, wqb_n, wkb_n))
                if (j + 1) % 2 == 0:
                    fillers += make_vproj_fillers(j + 1, wvb_n)
            fi = 0

            pts0 = [None] * 8
            for mc in range(8):
                qk_group(j, 0, mc, pts0)
                if mc == 2 and pending is not None:
                    norm_fin(pending[0], 1, pending[1])
                    pending = None
                if mc % 2 == 1 and fi < len(fillers):
                    fillers[fi]()
                    fi += 1
            if j == 0:
                # j=0 consumes its own fillers' outputs this iteration: the
                # nb1 projections (read by the nb1 QK groups below) and the
                # first two v-projection chains (vha m-chunks 0..3 + the
                # indicator columns, read by av_half(j,0,0)) must be emitted
                # before the nb1 phase; the last two v-chains follow between
                # the first nb1 groups, just ahead of av_half(j,0,1).
                while fi < 5:
                    fillers[fi]()
                    fi += 1
            pts1 = [None] * 8
            qk_group(j, 1, 0, pts1)
            av_half(j, 0, pts0, 0)
            if j == 0 and fi < len(fillers):
                fillers[fi]()   # v-chain for m-chunks 4,5
                fi += 1
            qk_group(j, 1, 1, pts1)
            if j == 0 and fi < len(fillers):
                fillers[fi]()   # v-chain for m-chunks 6,7
                fi += 1
            av_half(j, 0, pts0, 1)
            dens0 = norm_den(0)
            qk_group(j, 1, 2, pts1)
            qk_group(j, 1, 3, pts1)
            if fi < len(fillers):
                fillers[fi]()
                fi += 1
            qk_group(j, 1, 4, pts1)
            qk_group(j, 1, 5, pts1)
            norm_fin(j, 0, dens0)
            qk_group(j, 1, 6, pts1)
            qk_group(j, 1, 7, pts1)
            if j == NP - 1:
                # o-projection chains over the nb0 query rows only need
                # norm_fin(7, 0) (already emitted): they fill the PE while
                # the scalar engine computes the last exp batch.
                for nch in range(4):
                    for c2h in range(2):
                        oproj_chain(nch, c2h, evac=nc.vector)
            while fi < len(fillers):
                fillers[fi]()
                fi += 1
            av_half(j, 1, pts1, 0)
            av_half(j, 1, pts1, 1)
            pending = (j, norm_den(1))

        # ---- o-projection tail (+ bias) ----
        if pending is not None:
            norm_fin(pending[0], 1, pending[1])
            pending = None
        for nch in range(4, 8):
            for c2h in range(2):
                oproj_chain(nch, c2h)


def _get_nc():
    if "nc" not in _NC_CACHE:
        _NC_CACHE["nc"] = build_nc()
    return _NC_CACHE["nc"]


def _prep_inputs(q, k, v, mask, target_mask, Wq, Wk, Wv, Wo, bo):
    """Host-side staging: transpose + bf16-cast into exact device layouts."""
    q = np.asarray(q, np.float32)
    k = np.asarray(k, np.float32)
    v = np.asarray(v, np.float32)
    mask = np.asarray(mask, np.int32)
    target_mask = np.asarray(target_mask, np.int32)

    def t_layout(x):
        # [N, C] -> [p, cc, n] with value x[n, cc*128+p]
        xT = np.ascontiguousarray(x.T).astype(NPBF)
        return np.ascontiguousarray(xT.reshape(8, P, -1).transpose(1, 0, 2))

    Wqb = np.asarray(Wq, np.float32).astype(NPBF)
    Wkb = np.asarray(Wk, np.float32).astype(NPBF)
    Wvb = np.asarray(Wv, np.float32).astype(NPBF)
    Wob = np.asarray(Wo, np.float32).astype(NPBF)
    shared = {
        # wq[j, p, cc, dj] = Wq[cc*128+p, j*128+dj]
        "wq": np.ascontiguousarray(
            Wqb.reshape(8, P, NP, P).transpose(2, 1, 0, 3)),
        "wk": np.ascontiguousarray(
            Wkb.reshape(8, P, NP, P).transpose(2, 1, 0, 3)),
        # wv[jp, p, cc, dd] = Wv[cc*128+p, jp*256+dd]
        "wv": np.ascontiguousarray(
            Wvb.reshape(8, P, 4, 256).transpose(2, 1, 0, 3)),
        # wo[j, p, c2] = Wo[j*128+p, c2]
        "wo": np.ascontiguousarray(Wob.reshape(NP, P, C)),
        "bob": np.ascontiguousarray(np.concatenate(
            [np.asarray(bo, np.float32).astype(NPBF).reshape(1, C),
             np.zeros((P - 1, C), NPBF)], axis=0)),
        "eye": np.ascontiguousarray(np.eye(P, dtype=np.float32).astype(NPBF)),
    }
    in_maps = []
    for b in range(B):
        vm = v[b] * mask[b].astype(np.float32)[:, None]
        m = {
            "qbT": t_layout(q[b]),
            "kbT": t_layout(k[b]),
            "vbT": t_layout(vm),
            "tmT": t_layout(target_mask[b].astype(np.float32)),
            "maskb": np.ascontiguousarray(
                mask[b].astype(np.float32).astype(NPBF).reshape(8, P).T),
        }
        m.update(shared)
        in_maps.append(m)
    return in_maps


def kernel(q, k, v, mask, target_mask, Wq, Wk, Wv, Wo, bo):
    nc = _get_nc()
    in_maps = _prep_inputs(q, k, v, mask, target_mask, Wq, Wk, Wv, Wo, bo)
    res = bass_utils.run_bass_kernel_spmd(nc, in_maps, core_ids=list(range(B)))
    out = np.stack([res.results[b]["out"] for b in range(B)], axis=0)
    return out.astype(np.float32)


def run_traced(q, k, v, mask, target_mask, Wq, Wk, Wv, Wo, bo, **trace_kwargs):
    """Like kernel() but with NTFF tracing; returns (out, BassKernelResults)."""
    nc = _get_nc()
    in_maps = _prep_inputs(q, k, v, mask, target_mask, Wq, Wk, Wv, Wo, bo)
    res = bass_utils.run_bass_kernel_spmd(nc, in_maps, core_ids=list(range(B)),
                                          trace=True, **trace_kwargs)
    out = np.stack([res.results[b]["out"] for b in range(B)], axis=0)
    return out.astype(np.float32), res


# revision 46
# speedup vs baseline: 1.0004x; 1.0004x over previous
"""Trainium2 Bass kernel for batched multi-head attention (B=8, N=M=C=1024,
H=16, D=64), data-parallel across 8 NeuronCores (one batch element per core).

v4 (318us -> ~299us vs the v2 baseline): scheduling overhaul targeting PE
continuity and cross-engine queue latency:
  - startup DMAs round-robin across the sync/gpsimd/scalar sequencers in
    consumption order; j=1's q/k weights are prefetched so proj(1) chains
    serve as cheap early PE filler while the 2 MB of v lands (the startup
    window is DMA-bandwidth-bound at ~300 GB/s over 2-3 queues). First real
    matmul ~8.5us instead of ~26us; warmup trimmed 24->12 matmuls; the Exp
    activation table is preloaded during the DMA window.
  - QK regrouped to one m-chunk x 2 heads per [P,2,512] PSUM tile with the
    score ring deepened to bufs=3 (6 banks): ~3 groups in flight so the PE
    rides ahead of the scalar-engine exp without stalling on PSUM reuse.
    The den-broadcast matmuls ride the same ring (avpsum keeps only the two
    AV banks).
  - per-j khTz zero quadrants: j=0 on the DVE at startup, j>=1 lazily on
    gpsimd inside the k-projection fillers.
  - denominator evacuation on the DVE (gpsimd cannot access PSUM), so
    norm_fin's broadcast matmuls never queue behind the scalar exps;
    norm_fin placed ~4 groups after norm_den.
  - output DMA'd as bf16 (upcast on host): halves the tail DRAM write.
  - j=7's nb0 o-projection chains interleave before the last AV phase as PE
    cover for the final exp batch, with their PSUM evacuations on the DVE.

Per-core dataflow (bf16 matmul inputs, f32 PSUM accumulate): projections
with head-transposed layouts, QK^T with zero-padded K=128 lhsT, exp on the
scalar engine with 1/sqrt(D) folded in, target-mask as bf16 DVE multiplies,
AV with a 65th indicator column producing the softmax denominator in the
same accumulation chain, K=65 ones-matmul broadcast + fast reciprocal
normalize, o-projection with the bias folded in as a K=1 ones matmul.

Engine balance at ~299us: PE ~251us busy, Scalar (exp) ~242us, DVE ~217us.
"""
import sys

sys.path.insert(0, "/opt/trn_rl_repo")

import numpy as np
import ml_dtypes

import concourse.bass as bass  # noqa: F401
import concourse.mybir as mybir
import concourse.bacc as bacc
import concourse.tile as tile
from concourse import bass_utils

B = 8
N = 1024   # queries
M = 1024   # keys
C = 1024   # model dim
H = 16
D = 64
NP = 8     # head pairs
P = 128
NB = 2     # n blocks of 512
SCALE = D ** -0.5

F32 = mybir.dt.float32
F32R = mybir.dt.float32r
BF16 = mybir.dt.bfloat16
I32 = mybir.dt.int32
MUL = mybir.AluOpType.mult
EXP = mybir.ActivationFunctionType.Exp
NPBF = ml_dtypes.bfloat16

_NC_CACHE = {}


def build_nc():
    nc = bacc.Bacc("TRN2", target_bir_lowering=False, debug=False, num_devices=1)

    qbT_d = nc.dram_tensor("qbT", [P, 8, N], BF16, kind="ExternalInput").ap()
    kbT_d = nc.dram_tensor("kbT", [P, 8, M], BF16, kind="ExternalInput").ap()
    vbT_d = nc.dram_tensor("vbT", [P, 8, M], BF16, kind="ExternalInput").ap()
    tmT_d = nc.dram_tensor("tmT", [P, 8, N], BF16, kind="ExternalInput").ap()
    mb_d = nc.dram_tensor("maskb", [P, 8], BF16, kind="ExternalInput").ap()
    eye_d = nc.dram_tensor("eye", [P, P], BF16, kind="ExternalInput").ap()
    wq_d = nc.dram_tensor("wq", [NP, P, 8, P], BF16, kind="ExternalInput").ap()
    wk_d = nc.dram_tensor("wk", [NP, P, 8, P], BF16, kind="ExternalInput").ap()
    wv_d = nc.dram_tensor("wv", [4, P, 8, 256], BF16, kind="ExternalInput").ap()
    wo_d = nc.dram_tensor("wo", [NP, P, C], BF16, kind="ExternalInput").ap()
    bob_d = nc.dram_tensor("bob", [P, C], BF16, kind="ExternalInput").ap()
    out_d = nc.dram_tensor("out", [N, C], BF16, kind="ExternalOutput").ap()

    with tile.TileContext(nc) as tc:
        _body(tc, nc, qbT_d, kbT_d, vbT_d, tmT_d, mb_d, eye_d, wq_d, wk_d,
              wv_d, wo_d, bob_d, out_d)
    nc.compile()
    return nc


def _body(tc, nc, qbT_d, kbT_d, vbT_d, tmT_d, mb_d, eye_d, wq_d, wk_d, wv_d,
          wo_d, bob_d, out_d):
    from contextlib import ExitStack
    ctx = ExitStack()
    with ctx:
        persist = ctx.enter_context(tc.tile_pool(name="persist", bufs=1))
        wpool = ctx.enter_context(tc.tile_pool(name="wpool", bufs=2))
        ptpool = ctx.enter_context(tc.tile_pool(name="ptpool", bufs=8))
        xpool = ctx.enter_context(tc.tile_pool(name="xpool", bufs=2))
        opool = ctx.enter_context(tc.tile_pool(name="opool", bufs=2))
        spsum = ctx.enter_context(tc.tile_pool(name="spsum", bufs=3, space="PSUM"))
        avpsum = ctx.enter_context(tc.tile_pool(name="avpsum", bufs=2, space="PSUM"))

        # ---- persistent SBUF tensors ----
        qbT = persist.tile([P, 8, N], BF16)   # [p, cc, n] = q[n, cc*128+p]
        kbT = persist.tile([P, 8, M], BF16)
        vbT = persist.tile([P, 8, M], BF16)   # key-masked v, transposed
        tmT = persist.tile([P, 8, N], BF16)   # [p, mc, n] = tmask[n, mc*128+p]
        qhT = persist.tile([P, NP, N], BF16)  # [p, j, n] = qh[n, j*128+p]
        # khTz[:, j, 0, m]: rows 0:64 = kh^T head 2j, rows 64:128 = 0
        # khTz[:, j, 1, m]: rows 0:64 = 0, rows 64:128 = kh^T head 2j+1
        # Zero-padding makes every QK^T matmul a uniform K=128 config (same
        # PE tile config as the projections). Zero quadrants are written once
        # by a whole-tensor DVE memset below; the k-evacuations only fill the
        # nonzero quadrants.
        khTz = persist.tile([P, NP, 2, M], BF16)
        vha = persist.tile([P, NP, 8, 130], BF16)
        xn = persist.tile([P, NP, N], BF16)   # [p, j, n] = x_norm[n, j*128+p]
        wob = persist.tile([P, NP, C], BF16)  # [p, j, c2] = Wo[j*128+p, c2]
        maskb = persist.tile([P, 8], BF16)
        eyeb = persist.tile([P, P], BF16)  # identity: tail-chain PSUM reload
        bob = persist.tile([P, C], BF16)  # row 0 = bias, rows 1.. = 0
        # ones-row constants: lhsT of K-padded broadcast matmuls. Using a
        # 128-row lhsT keeps every matmul in the kernel at the same
        # (128, 128) PE tile config.
        onesr0 = persist.tile([P, P], BF16)   # row 0 = 1, rest 0
        nc.vector.memset(onesr0[:], 0.0)
        nc.vector.memset(onesr0[0:1, :], 1.0)
        wu = persist.tile([P, 512], BF16)
        nc.vector.memset(wu[:], 0.0)
        # j=0's khTz zero quadrants on the (otherwise idle) DVE during the
        # startup DMA window; j>=1 quadrants are zeroed lazily on gpsimd in
        # the k-projection fillers (the gpsimd queue is clear mid-loop but
        # backed up ~20us at startup behind the phase-1/2 DMA issue).
        nc.vector.memset(khTz[64:128, 0, 0, :], 0.0)
        nc.vector.memset(khTz[0:64, 0, 1, :], 0.0)

        # PE warmup: ramp the clock domain while the first DMAs land.
        wups = spsum.tile([P, 2, 512], F32, tag="sp", name="wups")
        for i in range(12):
            nc.tensor.matmul(wups[:, i % 2, :], wu[:, 0:P], wu[:],
                             start=True, stop=True)

        # ---- startup DMAs ----
        # Phase 1 (critical path: first q/k projection chains): j=0 weights +
        # the nb0 halves of q/k, round-robin across the three DMA-capable
        # sequencers (sync, gpsimd, scalar). A single sequencer takes ~600ns
        # per dma_start, so 24 descriptors on one queue would be ~14us of
        # issue latency; on three it's ~5us.
        wqb0 = wpool.tile([P, 8, P], BF16, tag="wq")
        wkb0 = wpool.tile([P, 8, P], BF16, tag="wk")
        wvb0 = wpool.tile([P, 8, 256], BF16, tag="wv")
        phase1 = []
        for i in range(4):
            cs = slice(2 * i, 2 * i + 2)
            phase1.append((wqb0[:, cs, :], wq_d[0, :, cs, :]))
            phase1.append((qbT[:, 2 * i, 0:512], qbT_d[:, 2 * i, 0:512]))
            phase1.append((qbT[:, 2 * i + 1, 0:512], qbT_d[:, 2 * i + 1, 0:512]))
        for i in range(4):
            cs = slice(2 * i, 2 * i + 2)
            phase1.append((wkb0[:, cs, :], wk_d[0, :, cs, :]))
            phase1.append((kbT[:, 2 * i, 0:512], kbT_d[:, 2 * i, 0:512]))
            phase1.append((kbT[:, 2 * i + 1, 0:512], kbT_d[:, 2 * i + 1, 0:512]))
        engs3 = [nc.sync, nc.gpsimd, nc.scalar]
        for i, (dst, src) in enumerate(phase1):
            engs3[i % 3].dma_start(out=dst, in_=src)
        # Preload the Exp activation table during the DMA window instead of
        # paying ~1.3us at the first real exp. Emitted AFTER the phase-1
        # descriptors (so it doesn't delay their issue on the scalar queue)
        # and targeting maskb (whose own DMA lands later and overwrites it)
        # rather than wu, to avoid a WAR wait on the warmup matmuls.
        nc.scalar.activation(maskb[0:1, 0:1], wu[0:1, 0:1], EXP, scale=1.0)

        # Phase 2, in consumption order: nb0 masks (first tm-mult ~14us),
        # nb1 q/k (the j=0 nb1 projection filler ~15us), Wv + the m-lower
        # half of v (first v-projection chain ~16us), the rest of q/k/v,
        # nb1 masks (~22us), then the stragglers. The first 12 descriptors
        # still use all three sequencers (the scalar queue clears well
        # before the first exp); the rest alternate sync/gpsimd.
        # j=1's q/k weights are prefetched here too: proj(1) chains are j=0's
        # cheapest early fillers (0.5 MB of weights buys 6.8us of PE work),
        # letting the 2 MB of vbT land later without stalling the PE.
        wqb1 = wpool.tile([P, 8, P], BF16, tag="wq")
        wkb1 = wpool.tile([P, 8, P], BF16, tag="wk")
        phase2 = [(maskb[:], mb_d)]
        for m2 in range(4):
            ms2 = slice(2 * m2, 2 * m2 + 2)
            phase2.append((tmT[:, ms2, 0:512], tmT_d[:, ms2, 0:512]))
        for cc in range(8):
            phase2.append((qbT[:, cc, 512:1024], qbT_d[:, cc, 512:1024]))
            phase2.append((kbT[:, cc, 512:1024], kbT_d[:, cc, 512:1024]))
        for i in range(4):
            cs = slice(2 * i, 2 * i + 2)
            phase2.append((wqb1[:, cs, :], wq_d[1, :, cs, :]))
            phase2.append((wkb1[:, cs, :], wk_d[1, :, cs, :]))
        for i in range(4):
            cs = slice(2 * i, 2 * i + 2)
            phase2.append((wvb0[:, cs, :], wv_d[0, :, cs, :]))
            phase2.append((vbT[:, 2 * i, 0:512], vbT_d[:, 2 * i, 0:512]))
            phase2.append((vbT[:, 2 * i + 1, 0:512], vbT_d[:, 2 * i + 1, 0:512]))
        for cc in range(8):
            phase2.append((vbT[:, cc, 512:1024], vbT_d[:, cc, 512:1024]))
        for m2 in range(4):
            ms2 = slice(2 * m2, 2 * m2 + 2)
            phase2.append((tmT[:, ms2, 512:1024], tmT_d[:, ms2, 512:1024]))
        phase2.append((wob[:, 0, :], wo_d[0]))
        phase2.append((wob[:, 1, :], wo_d[1]))
        phase2.append((bob[:], bob_d))
        phase2.append((eyeb[:], eye_d))
        engs2 = [nc.sync, nc.gpsimd]
        for i, (dst, src) in enumerate(phase2):
            if i < 18:
                engs3[i % 3].dma_start(out=dst, in_=src)
            else:
                engs2[i % 2].dma_start(out=dst, in_=src)

        def load_weights(j):
            """Weights for iteration j>=1, alternating sync/gpsimd."""
            wqb = wpool.tile([P, 8, P], BF16, tag="wq")
            wkb = wpool.tile([P, 8, P], BF16, tag="wk")
            descs = []
            for i in range(4):
                cs = slice(2 * i, 2 * i + 2)
                descs.append((wqb[:, cs, :], wq_d[j, :, cs, :]))
                descs.append((wkb[:, cs, :], wk_d[j, :, cs, :]))
            wvb = None
            if j % 2 == 0:
                wvb = wpool.tile([P, 8, 256], BF16, tag="wv")
                for i in range(4):
                    cs = slice(2 * i, 2 * i + 2)
                    descs.append((wvb[:, cs, :], wv_d[j // 2, :, cs, :]))
            descs.append((wob[:, j, :], wo_d[j]))
            for i, (dst, src) in enumerate(descs):
                engs2[i % 2].dma_start(out=dst, in_=src)
            return wqb, wkb, wvb

        # ---- helpers ----
        def proj_pair(j, nb, wqb, wkb):
            """q+k projection chains for (j, nb): 16 K=128 matmuls sharing
            one score-ring tile (emitted adjacently so the ring slot is
            held open only briefly), plus the PSUM->SBUF evacuations."""
            def run():
                sp = spsum.tile([P, 2, 512], F32, tag="sp", name=f"pj{j}_{nb}")
                ns = slice(nb * 512, (nb + 1) * 512)
                for cc in range(8):
                    nc.tensor.matmul(sp[:, 0, :], wqb[:, cc, :],
                                     qbT[:, cc, ns],
                                     start=(cc == 0), stop=(cc == 7))
                nc.vector.tensor_copy(qhT[:, j, ns], sp[:, 0, :])
                if nb == 0 and j > 0:
                    nc.gpsimd.memset(khTz[64:128, j, 0, :], 0.0)
                    nc.gpsimd.memset(khTz[0:64, j, 1, :], 0.0)
                for cc in range(8):
                    nc.tensor.matmul(sp[:, 1, :], wkb[:, cc, :],
                                     kbT[:, cc, ns],
                                     start=(cc == 0), stop=(cc == 7))
                nc.vector.tensor_copy(khTz[0:64, j, 0, ns], sp[0:64, 1, :])
                nc.vector.tensor_copy(khTz[64:128, j, 1, ns], sp[64:128, 1, :])
            return run

        def make_vproj_fillers(j, wvb):
            """v projection chains for pair (j, j+1), one filler per 2 m-chunks."""
            tiles = {}

            def chain(mg, mi2):
                def run():
                    if mg not in tiles:
                        tiles[mg] = spsum.tile([P, 2, 512], F32, tag="sp",
                                               name=f"pv{j}_{mg}")
                    pvv = tiles[mg].rearrange("p a b -> p (a b)").rearrange(
                        "p (m d) -> p m d", m=4)
                    if mg == 0 and mi2 == 0:
                        # indicator columns first (maskb lands early): every
                        # av chain reads them, so they must not wait for the
                        # last v-chain.
                        for jx in (j, j + 1):
                            nc.vector.tensor_copy(vha[:, jx, :, 64], maskb[:])
                            nc.vector.tensor_copy(vha[:, jx, :, 129], maskb[:])
                    for mi in (mi2, mi2 + 1):
                        mc = mg * 4 + mi
                        ms = slice(mc * P, (mc + 1) * P)
                        for cc in range(8):
                            nc.tensor.matmul(pvv[:, mi, :], vbT[:, cc, ms],
                                             wvb[:, cc, :],
                                             start=(cc == 0), stop=(cc == 7))
                        out_sl = vha[:, j:j + 2, mc, :].rearrange(
                            "p j (hx dd) -> p j hx dd", hx=2)[:, :, :, 0:64]
                        in_sl = pvv[:, mi, :].rearrange(
                            "p (j hx dd) -> p j hx dd", j=2, hx=2)
                        nc.vector.tensor_copy(out_sl, in_sl)
                return run

            return [chain(0, 0), chain(0, 2), chain(1, 0), chain(1, 2)]

        def qk_group(j, nb, mc, pts):
            """Scores for one m-chunk x 2 heads: 2 matmuls, 1 exp, 1 tm-mult."""
            ns = slice(nb * 512, (nb + 1) * 512)
            ms = slice(mc * P, (mc + 1) * P)
            sp = spsum.tile([P, 2, 512], F32, tag="sp")
            nc.tensor.matmul(sp[:, 0, :], khTz[:, j, 0, ms], qhT[:, j, ns],
                             start=True, stop=True)
            nc.tensor.matmul(sp[:, 1, :], khTz[:, j, 1, ms], qhT[:, j, ns],
                             start=True, stop=True)
            pt = ptpool.tile([P, 2, 512], BF16, tag="pt")
            nc.scalar.activation(pt[:], sp[:], EXP, scale=SCALE)
            tsl = tmT[:, mc, ns]
            nc.vector.tensor_tensor(pt[:, 0, :], pt[:, 0, :], tsl, MUL)
            nc.vector.tensor_tensor(pt[:, 1, :], pt[:, 1, :], tsl, MUL)
            pts[mc] = pt

        av_tiles = {}

        def av_half(j, nb, pts, half):
            """AV numerator+denominator accumulation, m-chunks half*4..half*4+3."""
            if half == 0:
                av_tiles[nb] = (avpsum.tile([65, 512], F32, tag="av", name="av0"),
                                avpsum.tile([65, 512], F32, tag="av", name="av1"))
            av0, av1 = av_tiles[nb]
            for mc in range(half * 4, half * 4 + 4):
                nc.tensor.matmul(av0[:], vha[:, j, mc, 0:65], pts[mc][:, 0, :],
                                 start=(mc == 0), stop=(mc == 7))
            for mc in range(half * 4, half * 4 + 4):
                nc.tensor.matmul(av1[:], vha[:, j, mc, 65:130], pts[mc][:, 1, :],
                                 start=(mc == 0), stop=(mc == 7))

        onesr64 = persist.tile([P, P], BF16)  # row 64 = 1, rest 0
        nc.vector.memset(onesr64[:], 0.0)
        nc.vector.memset(onesr64[64:65, :], 1.0)

        def norm_den(nb):
            """Phase 1: denominator rows -> bf16 SBUF. On the DVE (gpsimd
            cannot touch PSUM): its queue is shallower than the scalar
            engine's exp backlog, so norm_fin's bc matmuls aren't stuck
            behind head-of-line exps."""
            av0, av1 = av_tiles[nb]
            dens = []
            for av in (av0, av1):
                denb = xpool.tile([65, 512], BF16, tag="denb", bufs=4)
                nc.vector.tensor_copy(denb[:], av[:])
                dens.append(denb)
            return dens

        def norm_fin(j, nb, dens):
            """Phase 2: K=65 ones-matmul broadcast + reciprocal + multiply.
            One [P,2,512] score-ring tile holds both heads' broadcasts."""
            ns = slice(nb * 512, (nb + 1) * 512)
            av0, av1 = av_tiles[nb]
            bc2 = spsum.tile([P, 2, 512], F32, tag="sp", name="bc")
            nc.tensor.matmul(bc2[:, 0, :], onesr64[0:65, :],
                             dens[0][:], start=True, stop=True)
            nc.tensor.matmul(bc2[:, 1, :], onesr64[0:65, :],
                             dens[1][:], start=True, stop=True)
            for hx, av in enumerate((av0, av1)):
                rc = xpool.tile([64, 512], F32, tag="rc")
                nc.vector.reciprocal_approx_fast(rc[:], bc2[0:64, hx, :])
                rows = slice(0, 64) if hx == 0 else slice(64, 128)
                nc.vector.tensor_tensor(xn[rows, j, ns], av[0:64, :], rc[:], MUL)

        def oproj_evac_dma(nch, c2h, po, evac):
            ot = opool.tile([P, 512], BF16, tag="ot")
            if evac is nc.scalar:
                nc.scalar.copy(ot[:], po[:, 0, :])
            else:
                evac.tensor_copy(ot[:], po[:, 0, :])
            nsl = slice(nch * P, (nch + 1) * P)
            c0 = c2h * 512
            eng = nc.sync if (nch + c2h) % 2 == 0 else nc.gpsimd
            eng.dma_start(out=out_d[nsl, c0:c0 + 512], in_=ot[:])

        def oproj_chain(nch, c2h, evac=None):
            nsl = slice(nch * P, (nch + 1) * P)
            c2s = slice(c2h * 512, (c2h + 1) * 512)
            po = spsum.tile([P, 2, 512], F32, tag="sp", name="po")
            nc.tensor.matmul(po[:, 0, :], onesr0[:], bob[:, c2s],
                             start=True, stop=False)
            for jj in range(NP):
                nc.tensor.matmul(po[:, 0, :], xn[:, jj, nsl], wob[:, jj, c2s],
                                 start=False, stop=(jj == NP - 1))
            # bf16 output (upcast on host): halves the tail's DRAM write.
            oproj_evac_dma(nch, c2h, po, evac or nc.scalar)

        def oproj_partial(nch, c2h, store):
            """Tail chain, part 1: bias + head-pairs 0..6, evacuated to a
            bf16 staging slot. Runs as PE filler during j=7's nb1 phase so
            only a 2-matmul finish remains after the last normalize."""
            nsl = slice(nch * P, (nch + 1) * P)
            c2s = slice(c2h * 512, (c2h + 1) * 512)
            po = spsum.tile([P, 2, 512], F32, tag="sp", name="po")
            nc.tensor.matmul(po[:, 0, :], onesr0[:], bob[:, c2s],
                             start=True, stop=False)
            for jj in range(NP - 1):
                nc.tensor.matmul(po[:, 0, :], xn[:, jj, nsl], wob[:, jj, c2s],
                                 start=False, stop=(jj == NP - 2))
            nc.vector.tensor_copy(store, po[:, 0, :])

        def oproj_finish(nch, c2h, store):
            """Tail chain, part 2: identity-reload of the bf16 partial plus
            the head-pair-7 contribution."""
            nsl = slice(nch * P, (nch + 1) * P)
            c2s = slice(c2h * 512, (c2h + 1) * 512)
            po = spsum.tile([P, 2, 512], F32, tag="sp", name="po")
            nc.tensor.matmul(po[:, 0, :], eyeb[:], store,
                             start=True, stop=False)
            nc.tensor.matmul(po[:, 0, :], xn[:, NP - 1, nsl],
                             wob[:, NP - 1, c2s], start=False, stop=True)
            oproj_evac_dma(nch, c2h, po, nc.scalar)

        # ---- software-pipelined main loop ----
        # Pre-loop: only the nb0 projections for j=0; the nb1 chains and the
        # v-projections run as j=0's fillers so the PE is not stalled on
        # later-arriving DMAs.
        proj_pair(0, 0, wqb0, wkb0)()

        pending = None  # (j, dens) for the deferred nb1 normalize
        for j in range(NP):
            fillers = []
            if j == 0:
                fillers.append(proj_pair(0, 1, wqb0, wkb0))
                fillers.append(proj_pair(1, 0, wqb1, wkb1))
                fillers.append(proj_pair(1, 1, wqb1, wkb1))
                fillers += make_vproj_fillers(0, wvb0)
            elif j + 1 < NP:
                wqb_n, wkb_n, wvb_n = load_weights(j + 1)
                fillers.append(proj_pair(j + 1, 0, wqb_n, wkb_n))
                fillers.append(proj_pair(j + 1, 1, wqb_n, wkb_n))
                if (j + 1) % 2 == 0:
                    fillers += make_vproj_fillers(j + 1, wvb_n)
            fi = 0

            pts0 = [None] * 8
            for mc in range(8):
                qk_group(j, 0, mc, pts0)
                if mc == 2 and pending is not None:
                    norm_fin(pending[0], 1, pending[1])
                    pending = None
                if mc % 2 == 1 and fi < len(fillers):
                    fillers[fi]()
                    fi += 1
            if j == 0:
                # j=0 consumes its own fillers' outputs this iteration: the
                # nb1 projections (read by the nb1 QK groups below) and the
                # first two v-projection chains (vha m-chunks 0..3 + the
                # indicator columns, read by av_half(j,0,0)) must be emitted
                # before the nb1 phase; the last two v-chains follow between
                # the first nb1 groups, just ahead of av_half(j,0,1).
                while fi < 5:
                    fillers[fi]()
                    fi += 1
            pts1 = [None] * 8
            qk_group(j, 1, 0, pts1)
            av_half(j, 0, pts0, 0)
            if j == 0 and fi < len(fillers):
                fillers[fi]()   # v-chain for m-chunks 4,5
                fi += 1
            qk_group(j, 1, 1, pts1)
            if j == 0 and fi < len(fillers):
                fillers[fi]()   # v-chain for m-chunks 6,7
                fi += 1
            av_half(j, 0, pts0, 1)
            dens0 = norm_den(0)
            qk_group(j, 1, 2, pts1)
            qk_group(j, 1, 3, pts1)
            if fi < len(fillers):
                fillers[fi]()
                fi += 1
            qk_group(j, 1, 4, pts1)
            qk_group(j, 1, 5, pts1)
            norm_fin(j, 0, dens0)
            qk_group(j, 1, 6, pts1)
            qk_group(j, 1, 7, pts1)
            if j == NP - 1:
                # o-projection chains over the nb0 query rows only need
                # norm_fin(7, 0) (already emitted): they fill the PE while
                # the scalar engine computes the last exp batch.
                for nch in range(4):
                    for c2h in range(2):
                        oproj_chain(nch, c2h, evac=nc.vector)
            while fi < len(fillers):
                fillers[fi]()
                fi += 1
            av_half(j, 1, pts1, 0)
            av_half(j, 1, pts1, 1)
            pending = (j, norm_den(1))

        # ---- o-projection tail (+ bias) ----
        if pending is not None:
            norm_fin(pending[0], 1, pending[1])
            pending = None
        for nch in range(4, 8):
            for c2h in range(2):
                oproj_chain(nch, c2h)


def _get_nc():
    if "nc" not in _NC_CACHE:
        _NC_CACHE["nc"] = build_nc()
    return _NC_CACHE["nc"]


def _prep_inputs(q, k, v, mask, target_mask, Wq, Wk, Wv, Wo, bo):
    """Host-side staging: transpose + bf16-cast into exact device layouts."""
    q = np.asarray(q, np.float32)
    k = np.asarray(k, np.float32)
    v = np.asarray(v, np.float32)
    mask = np.asarray(mask, np.int32)
    target_mask = np.asarray(target_mask, np.int32)

    def t_layout(x):
        # [N, C] -> [p, cc, n] with value x[n, cc*128+p]
        xT = np.ascontiguousarray(x.T).astype(NPBF)
        return np.ascontiguousarray(xT.reshape(8, P, -1).transpose(1, 0, 2))

    Wqb = np.asarray(Wq, np.float32).astype(NPBF)
    Wkb = np.asarray(Wk, np.float32).astype(NPBF)
    Wvb = np.asarray(Wv, np.float32).astype(NPBF)
    Wob = np.asarray(Wo, np.float32).astype(NPBF)
    shared = {
        # wq[j, p, cc, dj] = Wq[cc*128+p, j*128+dj]
        "wq": np.ascontiguousarray(
            Wqb.reshape(8, P, NP, P).transpose(2, 1, 0, 3)),
        "wk": np.ascontiguousarray(
            Wkb.reshape(8, P, NP, P).transpose(2, 1, 0, 3)),
        # wv[jp, p, cc, dd] = Wv[cc*128+p, jp*256+dd]
        "wv": np.ascontiguousarray(
            Wvb.reshape(8, P, 4, 256).transpose(2, 1, 0, 3)),
        # wo[j, p, c2] = Wo[j*128+p, c2]
        "wo": np.ascontiguousarray(Wob.reshape(NP, P, C)),
        "bob": np.ascontiguousarray(np.concatenate(
            [np.asarray(bo, np.float32).astype(NPBF).reshape(1, C),
             np.zeros((P - 1, C), NPBF)], axis=0)),
        "eye": np.ascontiguousarray(np.eye(P, dtype=np.float32).astype(NPBF)),
    }
    in_maps = []
    for b in range(B):
        vm = v[b] * mask[b].astype(np.float32)[:, None]
        m = {
            "qbT": t_layout(q[b]),
            "kbT": t_layout(k[b]),
            "vbT": t_layout(vm),
            "tmT": t_layout(target_mask[b].astype(np.float32)),
            "maskb": np.ascontiguousarray(
                mask[b].astype(np.float32).astype(NPBF).reshape(8, P).T),
        }
        m.update(shared)
        in_maps.append(m)
    return in_maps


def kernel(q, k, v, mask, target_mask, Wq, Wk, Wv, Wo, bo):
    nc = _get_nc()
    in_maps = _prep_inputs(q, k, v, mask, target_mask, Wq, Wk, Wv, Wo, bo)
    res = bass_utils.run_bass_kernel_spmd(nc, in_maps, core_ids=list(range(B)))
    out = np.stack([res.results[b]["out"] for b in range(B)], axis=0)
    return out.astype(np.float32)


def run_traced(q, k, v, mask, target_mask, Wq, Wk, Wv, Wo, bo, **trace_kwargs):
    """Like kernel() but with NTFF tracing; returns (out, BassKernelResults)."""
    nc = _get_nc()
    in_maps = _prep_inputs(q, k, v, mask, target_mask, Wq, Wk, Wv, Wo, bo)
    res = bass_utils.run_bass_kernel_spmd(nc, in_maps, core_ids=list(range(B)),
                                          trace=True, **trace_kwargs)
    out = np.stack([res.results[b]["out"] for b in range(B)], axis=0)
    return out.astype(np.float32), res
